# revision 21
# baseline (speedup 1.0000x reference)
"""Distributed Trainium2 Bass kernel for GQA attention (B=2, S=2048, H=2048,
NH=16, NKV=4, HD=128) across 8 NeuronCores.

Sharding: core c -> (batch b = c//4, kv-group g = c%4).  Each core computes
Q/K/V projections for its 4 query heads + 1 kv head (column-sharded Wq/Wkv),
RoPE, causal flash-style attention in transposed layout (S^T = K Q^T so the
PV contraction lands on partitions), then AllGathers the per-group attention
outputs O^T across the 4 cores of its batch and applies a column shard of Wo
(full contraction, no all-reduce needed).  Output per core: y[b][:, 512g:512(g+1)].

All matmul operands are bf16 (1 cycle/row on PE); accumulation is f32 in PSUM;
softmax runs without max-subtraction (scores are ~N(0,1), exp is safe in f32).
Causal structure is exploited at column granularity: for a diagonal k-chunk at
offset d, only score columns >= d are computed/exp'd/accumulated, and the
staircase boundary is handled by one [128,128] triangle multiply.
"""

import math
import sys

sys.path.insert(0, "/opt/trn_rl_repo")

import numpy as np
import ml_dtypes

import concourse.bass as bass
import concourse.mybir as mybir
import concourse.tile as tile
from concourse import bacc
from concourse import bass_utils
from concourse.bass import ds, ts

BF16 = mybir.dt.bfloat16
F32 = mybir.dt.float32
AF = mybir.ActivationFunctionType

HD = 128      # head dim
GQ = 4        # query heads per core
QC = GQ * HD  # query columns per core (512)
SB = 512      # sequence block
P = 128


def build_kernel(S=2048, H=2048):
    NB = S // SB          # number of seq blocks
    HO = H // P           # hidden contraction chunks
    ST = SB // P          # seq tiles per block (4)
    OC = H // 4           # output columns per core
    NPAIR = 2             # head pairs per core (AG granularity)

    nc = bacc.Bacc("TRN2", target_bir_lowering=False, debug=False, num_devices=8)

    xt = nc.dram_tensor("xt", [H, S], BF16, kind="ExternalInput").ap()
    wq = nc.dram_tensor("wq", [H, QC], BF16, kind="ExternalInput").ap()
    wk = nc.dram_tensor("wk", [H, HD], BF16, kind="ExternalInput").ap()
    wv = nc.dram_tensor("wv", [H, HD], BF16, kind="ExternalInput").ap()
    wo = nc.dram_tensor("wo", [2048, OC], BF16, kind="ExternalInput").ap()
    cost = nc.dram_tensor("cost", [HD, S], F32, kind="ExternalInput").ap()
    sint = nc.dram_tensor("sint", [HD, S], F32, kind="ExternalInput").ap()
    rotm = nc.dram_tensor("rotm", [HD, HD], BF16, kind="ExternalInput").ap()
    ident = nc.dram_tensor("ident", [HD, HD], BF16, kind="ExternalInput").ap()
    trineg = nc.dram_tensor("trineg", [HD, HD], BF16, kind="ExternalInput").ap()
    out = nc.dram_tensor("out", [S, OC], F32, kind="ExternalOutput").ap()

    xt_r = xt.rearrange("(ho p) s -> p ho s", p=P)
    wq_r = wq.rearrange("(ho p) m -> p ho m", p=P)
    wk_r = wk.rearrange("(ho p) m -> p ho m", p=P)
    wv_r = wv.rearrange("(ho p) m -> p ho m", p=P)
    wo_r = wo.rearrange("(oc p) n -> p oc n", p=P)

    with tile.TileContext(nc) as tc:
        with (
            tc.tile_pool(name="consts", bufs=1) as consts,
            tc.tile_pool(name="wpool", bufs=1) as wpool,
            tc.tile_pool(name="xtp", bufs=2) as xtp,
            tc.tile_pool(name="kvp", bufs=1) as kvp,
            tc.tile_pool(name="qfp", bufs=2) as qfp,
            tc.tile_pool(name="work", bufs=3) as work,
            tc.tile_pool(name="ptp", bufs=6) as ptp,
            tc.tile_pool(name="gp", bufs=4) as gp,
            tc.tile_pool(name="psA", bufs=2, space="PSUM") as psA,
            tc.tile_pool(name="psS", bufs=3, space="PSUM") as psS,
            tc.tile_pool(name="psY", bufs=1, space="PSUM") as psY,
            tc.tile_pool(name="psO", bufs=2, space="PSUM") as psO,
            tc.tile_pool(name="dram", bufs=1, space="DRAM") as dpool,
        ):
            # ---- first-block activations + weights needed immediately ----
            # each contraction chunk is its own tile so consumers start as
            # soon as their chunk's DMA lands, not after whole-tile loads
            xt_tiles = []

            def load_xt(j):
                xt_sb = xtp.tile([P, HO, SB], BF16, name="xt_sb")
                for ho in range(HO):
                    nc.sync.dma_start(xt_sb[:, ho, :], xt_r[:, ho, ts(j, SB)])
                return xt_sb

            xt_tiles.append(load_xt(0))

            wq_chunks = []
            for ho in range(HO):
                wq_c = wpool.tile([P, QC], BF16, name=f"wq_c{ho}")
                nc.sync.dma_start(wq_c[:], wq_r[:, ho, :])
                wq_chunks.append(wq_c)
            rotm_sb = consts.tile([P, HD], BF16, name="rotm_sb")
            nc.sync.dma_start(rotm_sb[:], rotm[:])
            cos_tiles, sin_tiles = [], []
            for j in range(NB):
                cos_j = consts.tile([P, SB], F32, name=f"cos_{j}")
                sin_j = consts.tile([P, SB], F32, name=f"sin_{j}")
                cos_tiles.append(cos_j)
                sin_tiles.append(sin_j)
            nc.sync.dma_start(cos_tiles[0][:], cost[:, ts(0, SB)])
            nc.sync.dma_start(sin_tiles[0][:], sint[:, ts(0, SB)])
            wk_sb = wpool.tile([P, HO, HD], BF16, name="wk_sb")
            for ho in range(HO):
                nc.sync.dma_start(wk_sb[:, ho, :], wk_r[:, ho, :])
            wv_sb = wpool.tile([P, HO, HD], BF16, name="wv_sb")
            for ho in range(HO):
                nc.sync.dma_start(wv_sb[:, ho, :], wv_r[:, ho, :])
            for j in range(1, NB):
                nc.sync.dma_start(cos_tiles[j][:], cost[:, ts(j, SB)])
                nc.sync.dma_start(sin_tiles[j][:], sint[:, ts(j, SB)])
            ident_sb = consts.tile([P, HD], BF16, name="ident_sb")
            nc.sync.dma_start(ident_sb[:], ident[:])
            trineg_sb = consts.tile([P, HD], BF16, name="trineg_sb")
            nc.sync.dma_start(trineg_sb[:], trineg[:])
            ones_sb = consts.tile([P, 1], BF16, name="ones_sb")
            nc.vector.memset(ones_sb[:], 1.0)

            # K^T and V for the whole sequence (grow per block)
            kT_sb = kvp.tile([P, S], BF16, name="kT_sb")   # [hd, s]
            v_sb = kvp.tile([P, S], BF16, name="v_sb")     # [s%128, kc*128+hd]

            ag_ins = [[None] * NPAIR for _ in range(NB)]
            ag_outs = [[None] * NPAIR for _ in range(NB)]
            for j in range(NB):
                for pr in range(NPAIR):
                    ag_ins[j][pr] = dpool.tile(
                        [2 * P, SB], BF16, name=f"ag_in_{j}_{pr}")
                    ag_outs[j][pr] = dpool.tile(
                        [8 * P, SB], BF16, name=f"ag_out_{j}_{pr}")

            def rope(out_ap, ps_raw, j):
                """out = ps_raw*cos + (rot @ ps_raw)*sin, written as bf16."""
                q_raw = work.tile([P, SB], BF16, tag="qraw", name="q_raw")
                nc.vector.tensor_copy(q_raw[:], ps_raw[:])
                ps_rot = psA.tile([P, SB], F32, tag="ps", name="ps_rot")
                nc.tensor.matmul(ps_rot[:], rotm_sb[:], q_raw[:], start=True, stop=True)
                t1 = work.tile([P, SB], F32, tag="t1", name="t1")
                nc.vector.tensor_mul(t1[:], ps_raw[:], cos_tiles[j][:])
                t2 = work.tile([P, SB], F32, tag="t2", name="t2")
                nc.vector.tensor_mul(t2[:], ps_rot[:], sin_tiles[j][:])
                nc.vector.tensor_add(out_ap, t1[:], t2[:])

            def qkv_phase(j, xt_sb):
                q_all = qfp.tile([P, GQ, SB], BF16, name="q_all")
                for qc in range(GQ):
                    ps_q = psA.tile([P, SB], F32, tag="ps", name="ps_q")
                    for ho in range(HO):
                        nc.tensor.matmul(
                            ps_q[:], wq_chunks[ho][:, ts(qc, P)], xt_sb[:, ho, :],
                            start=(ho == 0), stop=(ho == HO - 1),
                        )
                    rope(q_all[:, qc, :], ps_q, j)
                ps_k = psA.tile([P, SB], F32, tag="ps", name="ps_k")
                for ho in range(HO):
                    nc.tensor.matmul(
                        ps_k[:], wk_sb[:, ho, :], xt_sb[:, ho, :],
                        start=(ho == 0), stop=(ho == HO - 1),
                    )
                rope(kT_sb[:, ts(j, SB)], ps_k, j)
                ps_v = psA.tile([P, SB], F32, tag="ps", name="ps_v")
                for st in range(ST):
                    for ho in range(HO):
                        nc.tensor.matmul(
                            ps_v[:, ts(st, P)], xt_sb[:, ho, ts(st, P)], wv_sb[:, ho, :],
                            start=(ho == 0), stop=(ho == HO - 1),
                        )
                nc.vector.tensor_copy(v_sb[:, ts(j, SB)], ps_v[:])
                return q_all

            def attn_head(j, q_all, h):
                """One head's causal attention for query block j."""
                KC = 4 * (j + 1)
                ps_o = psO.tile([P, SB], F32, tag="pso", name="ps_o")
                accs = [
                    work.tile([P, SB], BF16, tag="acca", name="acc_a"),
                    work.tile([P, SB], BF16, tag="accb", name="acc_b"),
                ]
                for kc in range(KC):
                    diag = kc >= 4 * j
                    d = P * (kc - 4 * j) if diag else 0
                    ps_s = psS.tile([P, SB], F32, tag="pss", name="ps_s")
                    nc.tensor.matmul(
                        ps_s[:, d:], kT_sb[:, ts(kc, P)], q_all[:, h, d:],
                        start=True, stop=not diag,
                    )
                    if diag:
                        # rank-128 update adds -40 on causally-masked slots;
                        # exp then yields ~0 with no vector-engine mask op
                        nc.tensor.matmul(
                            ps_s[:, d:d + P], ident_sb[:], trineg_sb[:],
                            start=False, stop=True,
                        )
                    pt = ptp.tile([P, SB], BF16, tag="pt", name="pt")
                    nc.scalar.activation(pt[:, d:], ps_s[:, d:], AF.Exp)
                    acc = accs[kc % 2]
                    if kc < 2:
                        if d > 0:
                            nc.vector.memset(acc[:, :d], 0.0)
                        nc.vector.tensor_copy(acc[:, d:], pt[:, d:])
                    else:
                        nc.vector.tensor_add(acc[:, d:], acc[:, d:], pt[:, d:])
                    nc.tensor.matmul(
                        ps_o[:, d:], v_sb[:, ts(kc, P)], pt[:, d:],
                        start=(kc == 0), stop=(kc == KC - 1),
                    )
                ps_d = psS.tile([1, SB], F32, tag="pss", name="ps_d")
                nc.tensor.matmul(ps_d[:], ones_sb[:], accs[0][:], start=True, stop=False)
                nc.tensor.matmul(ps_d[:], ones_sb[:], accs[1][:], start=False, stop=True)
                recip = work.tile([1, SB], F32, tag="recip", name="recip")
                nc.vector.reciprocal_approx_fast(recip[:], ps_d[:])
                rb = work.tile([P, SB], F32, tag="rb", name="rb")
                nc.gpsimd.partition_broadcast(rb[:], recip[:], channels=P)
                o_sb = work.tile([P, SB], BF16, tag="osb", name="o_sb")
                nc.vector.tensor_mul(o_sb[:], ps_o[:], rb[:])
                nc.sync.dma_start(ag_ins[j][h // 2][ts(h % 2, P), :], o_sb[:])

            def attn_phase(j, q_all):
                for h in range(GQ):
                    attn_head(j, q_all, h)
                    if h % 2 == 1:
                        pr = h // 2
                        nc.gpsimd.collective_compute(
                            "AllGather", mybir.AluOpType.bypass,
                            replica_groups=[[0, 1, 2, 3], [4, 5, 6, 7]],
                            ins=[ag_ins[j][pr][:].opt()],
                            outs=[ag_outs[j][pr][:].opt()],
                        )
                        wo_load(j, pr)

            g_loaded = {}

            def wo_load(j, pr):
                """Prefetch the gathered O^T chunks as soon as AG(j, pr) lands."""
                ag_r = ag_outs[j][pr].rearrange("(c p) s -> p c s", p=P)
                g_cs = []
                for c in range(8):
                    g_c = gp.tile([P, SB], BF16, tag=f"g{c}", name=f"g_c{c}")
                    nc.sync.dma_start(g_c[:], ag_r[:, c, :])
                    g_cs.append(g_c)
                g_loaded[(j, pr)] = g_cs

            def wo_phase(j):
                # per-pair partial contraction so the second AllGather's work
                # is the only thing left on the tail
                y_parts = [
                    work.tile([P, OC], F32, tag="ypart", name="y_part", bufs=4)
                    for _ in range(ST)
                ]
                for pr in range(NPAIR):
                    g_cs = g_loaded.pop((j, pr))
                    for st in range(ST):
                        ps_y = psY.tile([P, OC], F32, tag="psy", name="ps_y")
                        for c in range(8):
                            r, q = c // 2, c % 2
                            ocg = 4 * r + 2 * pr + q
                            nc.tensor.matmul(
                                ps_y[:], g_cs[c][:, ts(st, P)], wo_sb[:, ocg, :],
                                start=(c == 0), stop=(c == 7),
                            )
                        if pr == 0:
                            nc.vector.tensor_copy(y_parts[st][:], ps_y[:])
                        else:
                            y_sb = work.tile([P, OC], F32, tag="ysb", name="y_sb")
                            nc.vector.tensor_add(y_sb[:], y_parts[st][:], ps_y[:])
                            nc.sync.dma_start(
                                out[ds(j * SB + st * P, P), :], y_sb[:])

            # emission order: QKV(j+1) ahead of ATTN(j) so PE always has dense
            # projection work while ACT/DVE chew on attention; WO(j) lands
            # after ATTN(j+1).
            q_cur = qkv_phase(0, xt_tiles[0])
            if NB > 1:
                xt_tiles.append(load_xt(1))
            wo_sb = None
            for j in range(NB):
                q_next = None
                if j + 1 < NB:
                    q_next = qkv_phase(j + 1, xt_tiles[j + 1])
                    if j + 2 < NB:
                        xt_tiles.append(load_xt(j + 2))
                attn_phase(j, q_cur)
                q_cur = q_next
                if j == 0:
                    # wo weights are first needed by wo_phase(0)
                    wo_sb = wpool.tile([P, 16, OC], BF16, name="wo_sb")
                    for oc in range(16):
                        nc.sync.dma_start(wo_sb[:, oc, :], wo_r[:, oc, :])
                if j >= 1:
                    wo_phase(j - 1)
            wo_phase(NB - 1)

    return nc


def make_in_maps(x, cos, sin, Wq, Wkv, Wo, S=2048, H=2048):
    bf = ml_dtypes.bfloat16
    scale = 1.0 / math.sqrt(HD)
    NKVH = Wkv.shape[1] // (2 * HD)  # 4
    OC = H // 4

    Prot = np.zeros((HD, HD), np.float32)
    Prot[np.arange(64), np.arange(64) + 64] = -1.0
    Prot[np.arange(64) + 64, np.arange(64)] = 1.0
    rotm = np.ascontiguousarray(Prot.T).astype(bf)

    kk = np.arange(P)[:, None]
    w = np.arange(HD)[None, :]
    trineg_np = np.where(w < kk, -40.0, 0.0).astype(np.float32).astype(bf)
    ident_np = np.eye(HD, dtype=np.float32).astype(bf)

    cost = np.ascontiguousarray(cos.T).astype(np.float32)
    sint = np.ascontiguousarray(sin.T).astype(np.float32)

    in_maps = []
    for c in range(8):
        b, g = c // 4, c % 4
        in_maps.append({
            "xt": np.ascontiguousarray(np.asarray(x)[b].T).astype(bf),
            "wq": np.ascontiguousarray(np.asarray(Wq)[:, QC * g:QC * (g + 1)] * scale).astype(bf),
            "wk": np.ascontiguousarray(np.asarray(Wkv)[:, HD * g:HD * (g + 1)]).astype(bf),
            "wv": np.ascontiguousarray(
                np.asarray(Wkv)[:, NKVH * HD + HD * g:NKVH * HD + HD * (g + 1)]).astype(bf),
            "wo": np.ascontiguousarray(np.asarray(Wo)[:, OC * g:OC * (g + 1)]).astype(bf),
            "cost": cost, "sint": sint, "rotm": rotm,
            "ident": ident_np, "trineg": trineg_np,
        })
    return in_maps


_CACHE = {}


def _get_nc(S=2048, H=2048):
    key = (S, H)
    if key not in _CACHE:
        nc = build_kernel(S, H)
        nc.compile()
        _CACHE[key] = nc
    return _CACHE[key]


def run(x, cos, sin, Wq, Wkv, Wo, trace=False):
    S, H = 2048, 2048
    nc = _get_nc(S, H)
    in_maps = make_in_maps(x, cos, sin, Wq, Wkv, Wo, S, H)
    res = bass_utils.run_bass_kernel_spmd(
        nc, in_maps, core_ids=list(range(8)), trace=trace
    )
    OC = H // 4
    y = np.empty((2, S, H), np.float32)
    for c in range(8):
        b, g = c // 4, c % 4
        y[b][:, OC * g:OC * (g + 1)] = res.results[c]["out"]
    return y, res


def kernel(x, cos, sin, Wq, Wkv, Wo):
    y, _ = run(x, cos, sin, Wq, Wkv, Wo, trace=False)
    return y
